# revision 1
# baseline (speedup 1.0000x reference)
"""Multi-head self-attention Trainium2 kernel (8-core head-parallel).

Problem: B=2, N=2048, C=1024, H=16 heads, HD=64.
Sharding: tensor-parallel over heads -- each of the 8 cores computes 2 heads
(QKV slice + attention + partial output projection); the 8 partial projections
are summed on the host (unshard step), along with the projection bias.

All matmuls run as float32r (TF32-like, ~1.6e-4 rel err, full PE rate).
Device-side pipeline per core:
  1. qkv^T = w_loc^T @ x^T   (x^T prepared on host; contraction over C in
     8 chunks of 128), bias added during PSUM->SBUF evacuation (DVE).
  2. v^T re-transposed to natural [token, d] layout on the PE (identity
     matmul), with a constant 1.0 column appended per head so that the
     attn@v matmul also produces the softmax denominators as row 64.
  3. Per (batch, head): scores^T chunks [k=128, q=512] on PE, exp((1/8)s)
     on ACT straight out of PSUM (no max subtraction needed: |s| <~ 8),
     attn@v accumulation over 16 k-chunks into PSUM [65, 512].
  4. Normalization: reciprocal of row 64, partition-broadcast via SWDGE
     replicate DMA, multiply during evacuation (DVE).
  5. Partial projection out_part = oh @ w_proj[rows of this core's heads].
"""

import numpy as np

B, N, C = 2, 2048, 1024
H = 16
HD = C // H  # 64
SCALE = HD ** -0.5
T = B * N  # 4096 tokens
NCORES = 8
HPC = H // NCORES  # 2 heads per core

_CACHE = {}


def _build_program(phases=(1, 2, 3, 4), reps=1):
    import concourse.bass as bass
    import concourse.mybir as mybir
    import concourse.tile as tile
    from concourse import bacc

    f32 = mybir.dt.float32
    f32r = mybir.dt.float32r
    Exp = mybir.ActivationFunctionType.Exp
    Mult = mybir.AluOpType.mult

    nc = bacc.Bacc("TRN2", target_bir_lowering=False, debug=False,
                   num_devices=NCORES)

    xT_d = nc.dram_tensor("xT", [C, T], f32, kind="ExternalInput")
    wq_d = nc.dram_tensor("w_loc", [C, 3 * HPC * HD], f32, kind="ExternalInput")
    bq_d = nc.dram_tensor("b_loc", [128, 3], f32, kind="ExternalInput")
    w2_d = nc.dram_tensor("w2_loc", [HPC * HD, C], f32, kind="ExternalInput")
    id_d = nc.dram_tensor("ident", [128, 128], f32, kind="ExternalInput")
    ones_d = nc.dram_tensor("ones2", [128, 2], f32, kind="ExternalInput")
    ones64_d = nc.dram_tensor("ones64", [1, 64], f32, kind="ExternalInput")
    out_d = nc.dram_tensor("out_part", [T, C], f32, kind="ExternalOutput")

    CC = C // 128          # 8 contraction chunks
    NF = 3 * HPC * HD // 128   # 3 feature chunks (q, k, v)
    NTB = T // 512         # 8 token blocks
    NKC = N // 128         # 16 key chunks per batch
    NQB = N // 512         # 4 query blocks per batch
    NTC = T // 128         # 32 token chunks

    with tile.TileContext(nc) as tc:
        with tc.tile_pool(name="persist", bufs=1) as persist, \
             tc.tile_pool(name="xt", bufs=3, space="SBUF") as xt_pool, \
             tc.tile_pool(name="exp", bufs=4) as exp_pool, \
             tc.tile_pool(name="small", bufs=4) as small_pool, \
             tc.tile_pool(name="ob", bufs=3) as out_pool, \
             tc.tile_pool(name="ps", bufs=2, space="PSUM") as psum_s, \
             tc.tile_pool(name="aux", bufs=1, space="PSUM") as psum_aux, \
             tc.tile_pool(name="po", bufs=2, space="PSUM") as psum_o:

            w_sb = persist.tile([128, CC, 3 * HPC * HD], f32r, tag="w_sb")
            b_sb = persist.tile([128, 3], f32, tag="b_sb")
            w2_sb = persist.tile([128, C], f32r, tag="w2_sb")
            ident = persist.tile([128, 128], f32, tag="ident")
            qT = persist.tile([128, T], f32r, tag="qT")
            kT = persist.tile([128, T], f32r, tag="kT")
            vT = persist.tile([128, T], f32, tag="vT")
            # natural-layout v, per token-chunk: [vA(64) | 1 | vB(64) | 1]
            v_nat = persist.tile([128, NTC, 130], f32r, tag="v_nat")
            ohT = persist.tile([128, T], f32r, tag="ohT")

            # gpsimd DMAs cast f32 -> f32r (rounding in the SDMA datapath)
            nc.gpsimd.dma_start(
                out=w_sb[:],
                in_=wq_d[:].rearrange("(cc p) f -> p cc f", p=128))
            nc.gpsimd.dma_start(out=w2_sb[:], in_=w2_d[:])
            nc.sync.dma_start(out=ident[:], in_=id_d[:])
            nc.sync.dma_start(out=b_sb[:], in_=bq_d[:])
            ones64 = persist.tile([1, 64], f32r, tag="ones64")
            nc.gpsimd.dma_start(out=ones64[:], in_=ones64_d[:])

            qkvT = [qT, kT, vT]

            def v_nat_copy(pt, tcg):
                # single strided copy: pt cols [0:64],[64:128] land at
                # v_nat[:, tcg, 0:64] and [65:129] (skipping the ones col)
                src = pt[:, 0:128]
                dst = v_nat[:, tcg, 0:129]
                nc.vector.tensor_copy(
                    bass.AP(tensor=dst.tensor, offset=dst.offset,
                            ap=[list(dst.ap[0]), [65, 2], [1, 64]]),
                    bass.AP(tensor=src.tensor, offset=src.offset,
                            ap=[list(src.ap[0]), [64, 2], [1, 64]]))

            def emit_body(rep):
                # constant 1.0 columns (per-head softmax-denominator rows),
                # broadcast over token chunks from a tiny host input
                ones_ap = ones_d[:]
                for col, off in ((64, 0), (129, 1)) if 2 in phases else ():
                    nc.gpsimd.dma_start(
                        out=v_nat[:, :, col:col + 1],
                        in_=bass.AP(tensor=ones_ap.tensor, offset=off,
                                    ap=[[2, 128], [0, NTC], [1, 1]]))

                # ---- phase 1 (per batch): qkv^T = w_loc^T @ x^T, bias on
                # evac; v^T chunks transposed to natural layout as they land
                def emit_qkv(tb):
                    # one SWDGE cast-DMA per token block (f32 -> f32r)
                    xt = xt_pool.tile([128, CC, 512], f32r, tag="xt",
                                      name=f"xt_{rep}_{tb}")
                    nc.gpsimd.dma_start(
                        out=xt[:],
                        in_=xT_d[:, tb * 512:(tb + 1) * 512].rearrange(
                            "(cc p) t -> p cc t", p=128))
                    xts = [xt[:, ci, :] for ci in range(CC)]
                    for fc in range(NF):
                        ps = psum_s.tile([128, 512], f32, tag="s",
                                         name=f"ps1_{rep}_{tb}_{fc}")
                        for ci in range(CC):
                            nc.tensor.matmul(
                                ps[:],
                                w_sb[:, ci, fc * 128:(fc + 1) * 128],
                                xts[ci],
                                start=(ci == 0), stop=(ci == CC - 1))
                        nc.vector.tensor_scalar_add(
                            qkvT[fc][:, tb * 512:(tb + 1) * 512],
                            ps[:], b_sb[:, fc:fc + 1])
                    # phase 1.5 interleaved: transpose this block's v^T
                    for tcq in range(4) if 2 in phases else ():
                        tcg = tb * 4 + tcq
                        pt = psum_o.tile([128, 128], f32, tag="po",
                                         name=f"pt_{rep}_{tcg}")
                        sl = slice(tcg * 128, (tcg + 1) * 128)
                        nc.tensor.transpose(pt[:], vT[:, sl], ident[:])
                        v_nat_copy(pt, tcg)

                # ---- phase 2: attention per (batch, head) ----
                # score chunks for kc pairs share a 2-bank PSUM tile so one
                # ACT exp covers both; heads interleave for PE row-tiling
                def emit_attention(b):
                    for qb in range(NQB):
                        qsl = slice(b * N + qb * 512, b * N + (qb + 1) * 512)
                        po = [psum_o.tile([128, 512], f32, tag="po",
                                          name=f"po_{rep}_{b}_{qb}_{h}")
                              for h in range(HPC)]
                        for kcg in range(NKC // 2):
                            exs = {}
                            for h in range(HPC):
                                hsl = slice(h * 64, (h + 1) * 64)
                                ps = psum_s.tile(
                                    [128, 1024], f32, tag="s",
                                    name=f"ps2_{rep}_{b}_{qb}_{kcg}_{h}")
                                for kc2 in range(2):
                                    kc = kcg * 2 + kc2
                                    ksl = slice(b * N + kc * 128,
                                                b * N + (kc + 1) * 128)
                                    nc.tensor.matmul(
                                        ps[:, kc2 * 512:(kc2 + 1) * 512],
                                        kT[hsl, ksl], qT[hsl, qsl],
                                        start=True, stop=True)
                                ex = exp_pool.tile(
                                    [128, 1024], f32r, tag="ex",
                                    name=f"ex_{rep}_{b}_{qb}_{kcg}_{h}")
                                nc.scalar.activation(ex[:], ps[:], Exp,
                                                     scale=float(SCALE))
                                exs[h] = ex
                            for kc2 in range(2):
                                kc = kcg * 2 + kc2
                                tcg = b * NKC + kc
                                for h in range(HPC):
                                    nc.tensor.matmul(
                                        po[h][0:65, :],
                                        v_nat[:, tcg, h * 65:(h + 1) * 65],
                                        exs[h][:, kc2 * 512:(kc2 + 1) * 512],
                                        start=(kc == 0),
                                        stop=(kc == NKC - 1))
                        for h in range(HPC):
                            # broadcast sums row across partitions via a PE
                            # outer product (ones column x sums row), then
                            # reciprocal + multiply on DVE
                            s_sb = small_pool.tile(
                                [1, 512], f32r, tag="r",
                                name=f"s_sb_{rep}_{b}_{qb}_{h}")
                            nc.vector.tensor_copy(s_sb[:], po[h][64:65, :])
                            pr = psum_aux.tile([64, 512], f32, tag="aux",
                                               name=f"pr_{rep}_{b}_{qb}_{h}")
                            nc.tensor.matmul(pr[:], ones64[:], s_sb[:],
                                             start=True, stop=True)
                            rcp = small_pool.tile(
                                [64, 512], f32, tag="rb",
                                name=f"rcp_{rep}_{b}_{qb}_{h}")
                            nc.vector.reciprocal(rcp[:], pr[:])
                            nc.vector.tensor_tensor(
                                ohT[h * 64:(h + 1) * 64, qsl],
                                po[h][0:64, :], rcp[:], Mult)

                        # ---- phase 3 interleaved: project this q-block's
                        # 4 token chunks while the next q-block computes ----
                        for tcq in range(4) if 4 in phases else ():
                            tcg = b * 16 + qb * 4 + tcq
                            pp = psum_aux.tile([128, 1024], f32, tag="aux",
                                               name=f"pp_{rep}_{tcg}")
                            for jh in range(C // 512):
                                nc.tensor.matmul(
                                    pp[:, jh * 512:(jh + 1) * 512],
                                    ohT[:, tcg * 128:(tcg + 1) * 128],
                                    w2_sb[:, jh * 512:(jh + 1) * 512],
                                    start=True, stop=True)
                            ob = out_pool.tile([128, 1024], f32, tag="ob",
                                               name=f"ob_{rep}_{tcg}")
                            nc.vector.tensor_copy(ob[:], pp[:])
                            nc.sync.dma_start(
                                out=out_d[tcg * 128:(tcg + 1) * 128, :],
                                in_=ob[:])

                # per-batch orchestration: batch b's attention follows its
                # qkv blocks; the next batch's qkv fills attention bubbles
                for b in range(B):
                    if 1 in phases:
                        for tb in range(b * NTB // B, (b + 1) * NTB // B):
                            emit_qkv(tb)
                    if 3 in phases:
                        emit_attention(b)

            for rep in range(reps):
                emit_body(rep)

    nc.compile()
    return nc


def get_program():
    if "nc" not in _CACHE:
        _CACHE["nc"] = _build_program()
    return _CACHE["nc"]


def build_null_program():
    """Tiny kernel for calibrating per-dispatch overhead in test harnesses."""
    import concourse.mybir as mybir
    import concourse.tile as tile
    from concourse import bacc

    f32 = mybir.dt.float32
    nc = bacc.Bacc("TRN2", target_bir_lowering=False, debug=False,
                   num_devices=NCORES)
    x_in = nc.dram_tensor("x", [128, 128], f32, kind="ExternalInput")
    y_out = nc.dram_tensor("y", [128, 128], f32, kind="ExternalOutput")
    with tile.TileContext(nc) as tc:
        with tc.tile_pool(name="p", bufs=1) as pool:
            t = pool.tile([128, 128], f32)
            nc.sync.dma_start(out=t[:], in_=x_in[:])
            nc.sync.dma_start(out=y_out[:], in_=t[:])
    nc.compile()
    x = np.zeros((128, 128), dtype=np.float32)
    return nc, [{"x": x} for _ in range(NCORES)]


def make_in_maps(x, w_qkv, b_qkv, w_proj):
    """Host-side sharding: per-core input dicts."""
    xT = np.ascontiguousarray(x.reshape(T, C).T).astype(np.float32)
    ident = np.eye(128, dtype=np.float32)
    in_maps = []
    for core in range(NCORES):
        heads = [core * HPC + h for h in range(HPC)]
        # qkv feature columns for this core, ordered [qA qB kA kB vA vB]
        cols = []
        for s in range(3):  # q, k, v groups
            for h in heads:
                cols.append(np.arange(s * C + h * HD, s * C + (h + 1) * HD))
        cols = np.concatenate(cols)
        w_loc = np.ascontiguousarray(w_qkv[:, cols]).astype(np.float32)
        b_loc = np.ascontiguousarray(
            b_qkv[cols].reshape(3, HPC * HD).T).astype(np.float32)
        rows = np.concatenate(
            [np.arange(h * HD, (h + 1) * HD) for h in heads])
        w2_loc = np.ascontiguousarray(w_proj[rows, :]).astype(np.float32)
        in_maps.append({
            "xT": xT,
            "w_loc": w_loc,
            "b_loc": b_loc,
            "w2_loc": w2_loc,
            "ident": ident,
            "ones2": np.ones((128, 2), dtype=np.float32),
            "ones64": np.ones((1, 64), dtype=np.float32),
        })
    return in_maps


def combine_results(results, b_proj):
    """Host-side unshard: sum the 8 partial projections, add bias."""
    acc = np.zeros((T, C), dtype=np.float32)
    for res in results:
        acc += res["out_part"]
    acc += b_proj.astype(np.float32)[None, :]
    return acc.reshape(B, N, C)


def kernel(x, w_qkv, b_qkv, w_proj, b_proj):
    from concourse.bass_utils import run_bass_kernel_spmd

    x = np.asarray(x, dtype=np.float32)
    w_qkv = np.asarray(w_qkv, dtype=np.float32)
    b_qkv = np.asarray(b_qkv, dtype=np.float32)
    w_proj = np.asarray(w_proj, dtype=np.float32)
    b_proj = np.asarray(b_proj, dtype=np.float32)

    nc = get_program()
    in_maps = make_in_maps(x, w_qkv, b_qkv, w_proj)
    res = run_bass_kernel_spmd(nc, in_maps, list(range(NCORES)))
    return combine_results(res.results, b_proj)



# revision 11
# speedup vs baseline: 6.0395x; 6.0395x over previous
"""Multi-head self-attention Trainium2 kernel (8-core tensor-parallel).

Problem: B=2, N=2048, C=1024, H=16 heads, HD=64.

Sharding (v2 — minimizes per-call host<->device traffic AND HW time):
  - Inputs are token-sharded: core c receives only x^T columns for its 512
    tokens (bf16, 1 MB) plus its 2-head qkv weight slice and 128 rows of
    w_proj (bf16).  An on-device AllGather rebuilds the full x^T; a second
    AllGather rebuilds w_proj.
  - Attention runs head-parallel exactly like v1 (2 heads per core, full
    [N, N] scores, softmax denominators produced as row 64 of the attn@v
    accumulation via a constant-ones column appended to v).
  - An on-device AllToAll re-shards the attention output from head-major to
    token-major, so every core computes the FULL output projection for its
    own 512 tokens and writes a [512, C] fp32 slice.  No host-side reduce:
    the host just concatenates the 8 slices and adds b_proj.

All matmuls run in bf16 (fp32 PSUM accumulation): simulated end-to-end
max-rel error 5.7e-3 vs fp64 reference (gate 2e-2).
"""

import numpy as np

B, N, C = 2, 2048, 1024
H = 16
HD = C // H  # 64
SCALE = HD ** -0.5
T = B * N  # 4096 tokens
NCORES = 8
HPC = H // NCORES  # 2 heads per core
TS = T // NCORES   # 512 tokens per shard

_CACHE = {}


def _build_program():
    import concourse.bass as bass
    import concourse.mybir as mybir
    import concourse.tile as tile
    from concourse import bacc

    f32 = mybir.dt.float32
    f32r = mybir.dt.float32r
    bf16 = mybir.dt.bfloat16
    Exp = mybir.ActivationFunctionType.Exp
    Mult = mybir.AluOpType.mult

    nc = bacc.Bacc("TRN2", target_bir_lowering=False, debug=False,
                   num_devices=NCORES)

    xTs_d = nc.dram_tensor("xTs", [C, TS], bf16, kind="ExternalInput")
    wq_d = nc.dram_tensor("w_loc", [C, 3 * HPC * HD], bf16,
                          kind="ExternalInput")
    bq_d = nc.dram_tensor("b_loc", [128, 3], f32, kind="ExternalInput")
    w2s_d = nc.dram_tensor("w2s", [HPC * HD, C], bf16, kind="ExternalInput")
    id_d = nc.dram_tensor("ident", [128, 128], bf16, kind="ExternalInput")
    sel2_d = nc.dram_tensor("sel2", [128, 128], f32, kind="ExternalInput")
    out_d = nc.dram_tensor("out_s", [TS, C], f32, kind="ExternalOutput")

    CC = C // 128            # 8 contraction chunks over C
    NF = 3 * HPC * HD // 128  # 3 feature chunks (q, k, v)
    NTB = T // TS            # 8 token blocks (= NCORES)
    NKC = N // 128           # 16 key chunks per batch
    NQB = N // TS            # 4 query blocks per batch
    NTC = T // 128           # 32 token chunks
    RG = [list(range(NCORES))]

    with tile.TileContext(nc) as tc:
        with tc.tile_pool(name="persist", bufs=1) as persist, \
             tc.tile_pool(name="dram", bufs=1, space="DRAM") as dram, \
             tc.tile_pool(name="xt", bufs=3, space="SBUF") as xt_pool, \
             tc.tile_pool(name="exp", bufs=4) as exp_pool, \
             tc.tile_pool(name="small", bufs=4) as small_pool, \
             tc.tile_pool(name="ob", bufs=3) as out_pool, \
             tc.tile_pool(name="ps", bufs=2, space="PSUM") as psum_s, \
             tc.tile_pool(name="px", bufs=1, space="PSUM") as psum_x, \
             tc.tile_pool(name="po", bufs=2, space="PSUM") as psum_o:

            # ---- persistent SBUF tensors ----
            w_sb = persist.tile([128, CC, 3 * HPC * HD], bf16, tag="w_sb")
            b_sb = persist.tile([128, 3], f32, tag="b_sb")
            ident = persist.tile([128, 128], bf16, tag="ident")
            sel2 = persist.tile([128, 128], f32, tag="sel2")
            # denominator staging rows: head h's sums land at partition 64*h;
            # remaining partitions are zeroed once so the broadcast matmul
            # (sel2 has zero columns there) never multiplies garbage.
            s2 = persist.tile([128, 512], f32, tag="s2")
            nc.vector.memset(s2[:], 0.0)
            qT = persist.tile([128, T], bf16, tag="qT")
            kT = persist.tile([128, T], bf16, tag="kT")
            vT = persist.tile([128, T], bf16, tag="vT")
            # natural-layout v per token-chunk: [vA(64) | 1 | vB(64) | 1]
            v_nat = persist.tile([128, NTC, 130], bf16, tag="v_nat")
            ohT = persist.tile([128, T], bf16, tag="ohT")
            w2_sb = persist.tile([128, CC, C], bf16, tag="w2_sb")
            oh_all = persist.tile([128, NTB, TS], bf16, tag="oh_all")

            # ---- DRAM bounce buffers for collectives ----
            xg_in = dram.tile([C, TS], bf16)
            xg = dram.tile([C * NCORES, TS], bf16, addr_space="Shared")
            w2g_in = dram.tile([HPC * HD, C], bf16)
            w2g = dram.tile([HPC * HD * NCORES, C], bf16, addr_space="Shared")
            a2a_in = dram.tile([128 * NCORES, TS], bf16)
            a2a_out = dram.tile([128 * NCORES, TS], bf16)

            # ---- collectives: gather x^T shards and w_proj rows ----
            nc.sync.dma_start(out=xg_in[:], in_=xTs_d[:])
            nc.gpsimd.collective_compute(
                "AllGather", mybir.AluOpType.bypass, replica_groups=RG,
                ins=[xg_in[:]], outs=[xg[:]])
            nc.sync.dma_start(out=w2g_in[:], in_=w2s_d[:])
            nc.gpsimd.collective_compute(
                "AllGather", mybir.AluOpType.bypass, replica_groups=RG,
                ins=[w2g_in[:]], outs=[w2g[:]])

            # ---- constants ----
            nc.sync.dma_start(
                out=w_sb[:],
                in_=wq_d[:].rearrange("(cc p) f -> p cc f", p=128))
            nc.sync.dma_start(out=b_sb[:], in_=bq_d[:])
            nc.sync.dma_start(out=ident[:], in_=id_d[:])
            nc.sync.dma_start(out=sel2[:], in_=sel2_d[:])
            nc.sync.dma_start(
                out=w2_sb[:],
                in_=w2g[:].rearrange("(cc p) f -> p cc f", p=128))
            # ones columns for the softmax-denominator rows
            nc.vector.memset(v_nat[:, :, 64:65], 1.0)
            nc.vector.memset(v_nat[:, :, 129:130], 1.0)

            qkvT = [qT, kT, vT]

            def v_nat_copy(pt, tcg):
                # strided copy: pt cols [0:64],[64:128] -> v_nat cols
                # [0:64],[65:129] (skipping the ones column)
                src = pt[:, 0:128]
                dst = v_nat[:, tcg, 0:129]
                nc.vector.tensor_copy(
                    bass.AP(tensor=dst.tensor, offset=dst.offset,
                            ap=[list(dst.ap[0]), [65, 2], [1, 64]]),
                    bass.AP(tensor=src.tensor, offset=src.offset,
                            ap=[list(src.ap[0]), [64, 2], [1, 64]]))

            # ---- phase 1: qkv^T = w_loc^T @ x^T per token block ----
            def emit_qkv(tb):
                xt = xt_pool.tile([128, CC, TS], bf16, tag="xt",
                                  name=f"xt_{tb}")
                src = xg[tb * C:(tb + 1) * C, :]
                nc.sync.dma_start(
                    out=xt[:], in_=src.rearrange("(cc p) t -> p cc t", p=128))
                xts = [xt[:, ci, :] for ci in range(CC)]
                for fc in range(NF):
                    ps = psum_s.tile([128, 1024], f32, tag="s",
                                     name=f"ps1_{tb}_{fc}")
                    for ci in range(CC):
                        nc.tensor.matmul(
                            ps[:, 0:TS],
                            w_sb[:, ci, fc * 128:(fc + 1) * 128],
                            xts[ci],
                            start=(ci == 0), stop=(ci == CC - 1))
                    nc.vector.tensor_scalar_add(
                        qkvT[fc][:, tb * TS:(tb + 1) * TS],
                        ps[:, 0:TS], b_sb[:, fc:fc + 1])
                # transpose this block's v^T chunks to natural layout
                for tcq in range(TS // 128):
                    tcg = tb * (TS // 128) + tcq
                    pt = psum_o.tile([128, 1024], bf16, tag="po",
                                     name=f"pt_{tcg}")
                    sl = slice(tcg * 128, (tcg + 1) * 128)
                    nc.tensor.transpose(pt[:, 0:128], vT[:, sl], ident[:])
                    v_nat_copy(pt, tcg)

            # ---- phase 2: attention per (batch, query block) ----
            def emit_attention(b, qb):
                qsl = slice(b * N + qb * TS, b * N + (qb + 1) * TS)
                po = [psum_o.tile([128, 512], f32, tag="po",
                                  name=f"po_{b}_{qb}_{h}")
                      for h in range(HPC)]
                for kcg in range(NKC // 2):
                    exs = {}
                    for h in range(HPC):
                        hsl = slice(h * 64, (h + 1) * 64)
                        ps = psum_s.tile([128, 1024], f32, tag="s",
                                         name=f"ps2_{b}_{qb}_{kcg}_{h}")
                        for kc2 in range(2):
                            kc = kcg * 2 + kc2
                            ksl = slice(b * N + kc * 128,
                                        b * N + (kc + 1) * 128)
                            nc.tensor.matmul(
                                ps[:, kc2 * TS:(kc2 + 1) * TS],
                                kT[hsl, ksl], qT[hsl, qsl],
                                start=True, stop=True)
                        ex = exp_pool.tile([128, 1024], bf16, tag="ex",
                                           name=f"ex_{b}_{qb}_{kcg}_{h}")
                        nc.scalar.activation(ex[:], ps[:], Exp,
                                             scale=float(SCALE))
                        exs[h] = ex
                    for kc2 in range(2):
                        kc = kcg * 2 + kc2
                        tcg = b * NKC + kc
                        for h in range(HPC):
                            nc.tensor.matmul(
                                po[h][0:65, :],
                                v_nat[:, tcg, h * 65:(h + 1) * 65],
                                exs[h][:, kc2 * TS:(kc2 + 1) * TS],
                                start=(kc == 0),
                                stop=(kc == NKC - 1))
                # normalization: reciprocal of denominator row, broadcast
                # to both heads' 64 partitions via one PE outer product
                for h in range(HPC):
                    nc.vector.tensor_copy(s2[h * 64:h * 64 + 1, :],
                                          po[h][64:65, :])
                prd = psum_x.tile([128, 512], f32, tag="aux",
                                  name=f"prd_{b}_{qb}")
                nc.tensor.matmul(prd[:], sel2[:], s2[:],
                                 start=True, stop=True)
                rcp = small_pool.tile([128, 512], f32, tag="rb",
                                      name=f"rcp_{b}_{qb}")
                nc.vector.reciprocal_approx_fast(out=rcp[:], in_=prd[:])
                for h in range(HPC):
                    nc.vector.tensor_tensor(
                        ohT[h * 64:(h + 1) * 64, qsl],
                        po[h][0:64, :], rcp[h * 64:(h + 1) * 64, :], Mult)

            # ---- emission: batch 0 qkv, batch 0 attention interleaved
            # with batch 1 qkv, batch 1 attention ----
            for tb in range(NTB // B):
                emit_qkv(tb)
            for qb in range(NQB):
                emit_attention(0, qb)
            for tb in range(NTB // B, NTB):
                emit_qkv(tb)
            for qb in range(NQB):
                emit_attention(1, qb)

            # ---- AllToAll: head-major ohT -> token-major oh_all ----
            nc.sync.dma_start(
                out=a2a_in[:].rearrange("(blk p) t -> p blk t", p=128),
                in_=ohT[:].rearrange("p (blk t) -> p blk t", blk=NTB))
            nc.gpsimd.collective_compute(
                "AllToAll", mybir.AluOpType.bypass, replica_groups=RG,
                ins=[a2a_in[:]], outs=[a2a_out[:]])
            nc.sync.dma_start(
                out=oh_all[:],
                in_=a2a_out[:].rearrange("(blk p) t -> p blk t", p=128))

            # ---- phase 3: full output projection for own 512 tokens ----
            for tcq in range(TS // 128):
                pp = psum_s.tile([128, 1024], f32, tag="s",
                                 name=f"pp_{tcq}")
                for jh in range(C // 512):
                    for r in range(NCORES):
                        nc.tensor.matmul(
                            pp[:, jh * 512:(jh + 1) * 512],
                            oh_all[:, r, tcq * 128:(tcq + 1) * 128],
                            w2_sb[:, r, jh * 512:(jh + 1) * 512],
                            start=(r == 0), stop=(r == NCORES - 1))
                ob = out_pool.tile([128, 1024], f32, tag="ob",
                                   name=f"ob_{tcq}")
                nc.vector.tensor_copy(ob[:], pp[:])
                nc.sync.dma_start(
                    out=out_d[tcq * 128:(tcq + 1) * 128, :],
                    in_=ob[:])

    nc.compile()
    return nc


def get_program():
    if "nc" not in _CACHE:
        _CACHE["nc"] = _build_program()
    return _CACHE["nc"]


def build_null_program():
    """Tiny kernel for calibrating per-dispatch overhead in test harnesses."""
    import concourse.mybir as mybir
    import concourse.tile as tile
    from concourse import bacc

    f32 = mybir.dt.float32
    nc = bacc.Bacc("TRN2", target_bir_lowering=False, debug=False,
                   num_devices=NCORES)
    x_in = nc.dram_tensor("x", [128, 128], f32, kind="ExternalInput")
    y_out = nc.dram_tensor("y", [128, 128], f32, kind="ExternalOutput")
    with tile.TileContext(nc) as tc:
        with tc.tile_pool(name="p", bufs=1) as pool:
            t = pool.tile([128, 128], f32)
            nc.sync.dma_start(out=t[:], in_=x_in[:])
            nc.sync.dma_start(out=y_out[:], in_=t[:])
    nc.compile()
    x = np.zeros((128, 128), dtype=np.float32)
    return nc, [{"x": x} for _ in range(NCORES)]


def make_in_maps(x, w_qkv, b_qkv, w_proj):
    """Host-side sharding: per-core input dicts (bf16 weights/activations)."""
    import ml_dtypes
    bf16 = ml_dtypes.bfloat16

    xT = np.ascontiguousarray(
        x.reshape(T, C).T.astype(bf16))
    ident = np.eye(128, dtype=bf16)
    sel2 = np.zeros((128, 128), dtype=np.float32)
    for h in range(HPC):
        sel2[h * 64, h * 64:(h + 1) * 64] = 1.0
    in_maps = []
    for core in range(NCORES):
        heads = [core * HPC + h for h in range(HPC)]
        cols = []
        for s in range(3):  # q, k, v groups
            for h in heads:
                cols.append(np.arange(s * C + h * HD, s * C + (h + 1) * HD))
        cols = np.concatenate(cols)
        w_loc = np.ascontiguousarray(w_qkv[:, cols].astype(bf16))
        b_loc = np.ascontiguousarray(
            b_qkv[cols].reshape(3, HPC * HD).T).astype(np.float32)
        rows = np.concatenate(
            [np.arange(h * HD, (h + 1) * HD) for h in heads])
        w2s = np.ascontiguousarray(w_proj[rows, :].astype(bf16))
        in_maps.append({
            "xTs": np.ascontiguousarray(xT[:, core * TS:(core + 1) * TS]),
            "w_loc": w_loc,
            "b_loc": b_loc,
            "w2s": w2s,
            "ident": ident,
            "sel2": sel2,
        })
    return in_maps


def combine_results(results, b_proj):
    """Host-side unshard: concatenate the 8 token slices, add bias."""
    acc = np.concatenate(
        [np.asarray(res["out_s"], dtype=np.float32) for res in results],
        axis=0)
    acc += b_proj.astype(np.float32)[None, :]
    return acc.reshape(B, N, C)


def kernel(x, w_qkv, b_qkv, w_proj, b_proj):
    from concourse.bass_utils import run_bass_kernel_spmd

    x = np.asarray(x, dtype=np.float32)
    w_qkv = np.asarray(w_qkv, dtype=np.float32)
    b_qkv = np.asarray(b_qkv, dtype=np.float32)
    w_proj = np.asarray(w_proj, dtype=np.float32)
    b_proj = np.asarray(b_proj, dtype=np.float32)

    nc = get_program()
    in_maps = make_in_maps(x, w_qkv, b_qkv, w_proj)
    res = run_bass_kernel_spmd(nc, in_maps, list(range(NCORES)))
    return combine_results(res.results, b_proj)
